# revision 3
# baseline (speedup 1.0000x reference)
"""Trainium2 Bass kernel for nn_BertEmbedding_1623497638029.

Segment-mean of BERT subword embeddings into token embeddings:

    out[b, t] = mean(enc[b, lo_t : lo_t + lens[b, t]]),  0 if lens == 0

Design (PE-routing): the host precomputes, per core, a sparse routing
matrix M with the mean weights folded in (M[p, t] = 1/lens[b,t] iff
subword row p belongs to token t, 0 otherwise) and per-row gather
indices for the partially-valid upper half of the subword axis.
Device steady state, per batch row:

  - rows 0..255 (always valid: totals >= 291 for every batch row): one
    contiguous read triggered on the Activation engine's HWDGE queue
  - rows 256..383 / 384..511: two single-index-per-partition indirect
    gathers (SWDGE); rows >= totals carry OOB indices and are skipped
    by the bounds check, so only the ~67% valid part of enc is read
  - 16 float32r matmuls (full PE rate at moving dim >= 256) accumulate
    the token outputs over the 4 subword chunks into PSUM
  - PSUM -> SBUF copies split across the Activation + Vector engines
  - one contiguous 768 KB store per batch row on the SP HWDGE queue

f32r reduced-precision matmul keeps rel err ~1e-4 (correctness gate is
2e-2).  Gather buffers are primed once with real enc data so OOB-skipped
descriptors never leave NaN bit patterns for the PE (their M columns are
exactly zero, so the values themselves are irrelevant).

Sharding: pure data parallel - 8 batch rows per NeuronCore, no
cross-core communication.

Measured steady state: ~46.3 us/iteration per core (vs 66.0 us for the
previous window-gather + vector-combine kernel) - DMA-bound at ~320 GB/s
effective on 14.7 MB/iteration of HBM traffic.
"""

import numpy as np

import concourse.bacc as bacc
import concourse.bass as bass
import concourse.mybir as mybir
import concourse.tile as tile
from concourse.bass_utils import run_bass_kernel_spmd

NCORES = 8
BZ, P, T, H = 64, 512, 256, 768
BL = BZ // NCORES  # batch rows per core

F32 = mybir.dt.float32
F32R = mybir.dt.float32r
I32 = mybir.dt.int32
AF = mybir.ActivationFunctionType


def precompute(lens_np):
    """Host-side: per-core routing matrices and gather indices.

    M[k, (b,c,t)] = 1/lens[b,t] iff global subword row c*128+k of batch
    row b belongs to token t; gidx[k, (b,c)] = enc row index for
    partition k of chunk c (OOB sentinel P*BL for padding rows).
    """
    lens64 = lens_np.astype(np.int64)
    cum = np.cumsum(lens64, axis=1)                                  # (BZ,T)
    totals = cum[:, -1]
    # seg[b, p] = searchsorted(cum[b], p, 'right') = #{t: cum[t] <= p}
    seg = (cum[:, None, :] <= np.arange(P)[None, :, None]).sum(-1)   # (BZ,P)
    w = (1.0 / np.maximum(lens64, 1)).astype(np.float32)
    onehot = seg[:, :, None] == np.arange(T)[None, None, :]          # (BZ,P,T)
    mfull = onehot * w[:, None, :]
    k = np.arange(128)
    rowmap = np.stack([k, 128 + k, 256 + k, 384 + k])                # (4,128)
    ms, gs = [], []
    for i in range(NCORES):
        r0 = i * BL
        mcore = (
            mfull[r0 : r0 + BL][:, rowmap, :]                        # (BL,4,128,T)
            .transpose(2, 0, 1, 3)
            .reshape(128, BL * 4 * T)
            .astype(np.float32)
        )
        tot = totals[r0 : r0 + BL]
        valid = rowmap[None] < tot[:, None, None]                    # (BL,4,128)
        gi = np.where(
            valid, np.arange(BL)[:, None, None] * P + rowmap[None], P * BL
        ).astype(np.int32)
        gs.append(np.ascontiguousarray(gi.transpose(2, 0, 1).reshape(128, BL * 4)))
        ms.append(np.ascontiguousarray(mcore))
    return ms, gs


def _build_nc(repeat=0, timing=False, gbufs=4, pbufs=4, sbufs=3):
    """timing=True keeps only gidx as ExternalInput (enc/md Internal, tiny
    dummy output) so repeat-loop wall-clock deltas are nearly noise-free."""
    nc = bacc.Bacc(
        "TRN2", target_bir_lowering=False, debug=False,
        num_devices=NCORES, enable_asserts=True,
    )
    big_in = "Internal" if timing else "ExternalInput"
    big_out = "Internal" if timing else "ExternalOutput"
    enc = nc.dram_tensor("enc", [BL * P, H], F32R, kind=big_in).ap()
    gidx = nc.dram_tensor("gidx", [128, 4 * BL], I32, kind="ExternalInput").ap()
    md = nc.dram_tensor("md", [128, BL * 4 * T], F32R, kind=big_in).ap()
    out = nc.dram_tensor("out", [BL * T, H], F32, kind=big_out).ap()
    dum = (nc.dram_tensor("dum", [1, 8], I32, kind="ExternalOutput").ap()
           if timing else None)

    with tile.TileContext(nc) as tc:
        with (
            tc.tile_pool(name="const", bufs=1) as cpool,
            tc.tile_pool(name="g", bufs=gbufs) as gpool,
            tc.tile_pool(name="psum", bufs=pbufs, space="PSUM") as ppool,
            tc.tile_pool(name="stage", bufs=sbufs) as spool,
        ):
            gidx_sb = cpool.tile([128, 4 * BL], I32)
            nc.sync.dma_start(out=gidx_sb[:], in_=gidx[:, :])
            if timing:
                nc.sync.dma_start(out=dum[:, :], in_=gidx_sb[0:1, 0:8])
            M_all = cpool.tile([128, BL * 4 * T], F32R)
            nc.sync.dma_start(out=M_all[:], in_=md[:, :])
            # prime gather buffers with finite data (f32r memset is not a
            # valid ISA op); OOB-skipped rows then never contain NaN bits
            for _ in range(gbufs):
                t = gpool.tile([128, 4 * H], F32R, tag="g")
                nc.sync.dma_start(
                    out=t[:].rearrange("p (c h) -> p c h", h=H),
                    in_=enc[0:P, :].rearrange("(c p) h -> p c h", p=128))

            def body(_iv=None):
                for b in range(BL):
                    g = gpool.tile([128, 4 * H], F32R, tag="g")
                    # chunks 0-1: contiguous read (Act HWDGE queue).
                    # chunks 2-3: one-index-per-partition gathers; the HW
                    # transfers the dest free extent per index, and OOB
                    # indices (padding rows) are skipped entirely.
                    nc.scalar.dma_start(
                        out=g[:, 0 : 2 * H].rearrange("p (c h) -> p c h", h=H),
                        in_=enc[b * P : b * P + 256, :].rearrange(
                            "(c p) h -> p c h", p=128),
                    )
                    for c in (2, 3):
                        nc.gpsimd.indirect_dma_start(
                            out=g[:, c * H : (c + 1) * H], out_offset=None,
                            in_=enc[:, :],
                            in_offset=bass.IndirectOffsetOnAxis(
                                ap=gidx_sb[:, 4 * b + c : 4 * b + c + 1],
                                axis=0),
                            bounds_check=BL * P - 1, oob_is_err=False,
                        )
                    st = spool.tile([128, 2 * H], F32, tag="st")
                    for tc_ in range(2):
                        base = (b * 4) * T + tc_ * 128
                        for half in range(2):
                            # [128,512] = one PSUM bank; the [0:384] matmul
                            # output never crosses a bank boundary
                            ps = ppool.tile([128, 512], F32, tag=f"ps{half}")
                            for c in range(4):
                                nc.tensor.matmul(
                                    out=ps[:, 0 : H // 2],
                                    lhsT=M_all[:, base + c * T : base + c * T
                                               + 128],
                                    rhs=g[:, c * H + half * (H // 2) :
                                          c * H + (half + 1) * (H // 2)],
                                    start=(c == 0), stop=(c == 3),
                                )
                            dst = st[:, tc_ * H + half * (H // 2) :
                                     tc_ * H + (half + 1) * (H // 2)]
                            if tc_ == 0:
                                nc.scalar.activation(
                                    out=dst, in_=ps[:, 0 : H // 2],
                                    func=AF.Copy)
                            else:
                                nc.vector.tensor_copy(dst, ps[:, 0 : H // 2])
                    dest = out[b * T : (b + 1) * T, :].rearrange(
                        "(c t) h -> t c h", c=2)
                    nc.sync.dma_start(
                        out=dest, in_=st[:].rearrange("p (c h) -> p c h", h=H))

            if repeat:
                tc.For_i_unrolled(0, repeat, 1, body, max_unroll=2)
            else:
                body()

    nc.compile()
    return nc


_NC = None


def _get_nc():
    global _NC
    if _NC is None:
        _NC = _build_nc()
    return _NC


def kernel(enc_out, bert_mask, bert_lens):
    del bert_mask  # implied by bert_lens (mask = arange(P) < totals)
    enc_np = np.ascontiguousarray(np.asarray(enc_out, dtype=np.float32))
    lens_np = np.ascontiguousarray(np.asarray(bert_lens, dtype=np.int32))
    assert enc_np.shape == (BZ, P, H) and lens_np.shape == (BZ, T)

    ms, gs = precompute(lens_np)
    nc = _get_nc()
    in_maps = [
        {
            "enc": enc_np[i * BL : (i + 1) * BL].reshape(BL * P, H),
            "gidx": gs[i],
            "md": ms[i],
        }
        for i in range(NCORES)
    ]
    results = run_bass_kernel_spmd(nc, in_maps, core_ids=list(range(NCORES))).results
    out = np.concatenate([r["out"] for r in results], axis=0)
    return out.reshape(BZ, T, H)
